# revision 8
# baseline (speedup 1.0000x reference)
"""Trainium2 Bass kernel for block-neighbor "contamination" stencil.

Problem: x [B=8, C=32, H=512, W=512] f32, kernel_size k=8.
The image is a 64x64 grid of 8x8 blocks. For each block, out = 0.8*block +
0.2 * mean(8 neighboring blocks) elementwise over the 8x8 tile, with
zero-padding of the block grid and per-position valid-neighbor counts
(interior 8, edges 5, corners 3).

Equivalent pixel form: a sparse 3x3 stencil with taps at +-8 pixels:
    out[r,w] = 0.8*x[r,w] + beta(r,w) * nsum[r,w]
    nsum[r,w] = sum over (dr,dw) in {-8,0,8}^2, (dr,dw) != (0,0), of
                x[r+dr, w+dw]  (zero pad at image borders)
    beta(r,w) = 0.2 / count(r,w),  count = Nr*Nw - 1,
    Nr/Nw = 2 at the first/last block row/col, else 3.

Strategy (pure data parallel, 1 batch item per NeuronCore, 8 cores):
  * The kernel is HBM-bandwidth bound (~358 GB/s/core), so all HBM I/O is
    fp16: the host casts x to fp16 before upload and widens y back to f32
    after download. That halves DRAM traffic vs f32 (32 MiB/core total)
    and costs ~5e-4 relative error - far inside the 2e-2 gate.
  * Layout: SBUF partition p = (channel-pair, block-row bi); free dim =
    (u = row-within-block 0..7, w 0..511). One partition = one block-row =
    8 consecutive image rows; a 128-partition chunk is one contiguous
    1 MiB DRAM region.
  * Work is split so no compute engine exceeds ~75us (under the ~100us
    DMA floor):
      - DVE precomputes the horizontal pair-sum hp[w] = x[w-8] + x[w+8]
        in fp16 (2x DVE mode), plus 1/4 of the PSUM->fp16 downcast.
      - PE does 2 banded matmuls per u-slice: wv = 0.8*I + beta*band2
        (vertical taps, partition +-1, block-diagonal per channel) on x,
        and wh = beta*band3 on hp. PSUM accumulates the final value.
      - ScalarE writes hp's 8-wide edge columns as gamma(partition)-
        scaled copies and does 3/4 of the downcast.
  * Block-column edges (first/last 8 columns): true coefficient is
    gamma*beta, handled by (a) gamma-prescaling hp's edge columns and
    (b) a tiny wcor = (gamma-1)*beta*band2 matmul on x's edge columns.
    gamma varies per OUTPUT partition but hp prescaling uses the SOURCE
    partition's gamma; they differ only for cross-block-row taps at
    block-rows 0/1/62/63 (1.6 vs 5/3), adding ~1e-4 norm error. Total
    relative error stays ~6e-4, far inside the 2e-2 gate.
  * Input DMAs ride the qSyncDynamicHW ring, output DMAs the
    qScalarDynamicHW ring, so loads and stores stream concurrently.

HBM traffic per core: 16 MiB in + 16 MiB out @ ~358 GB/s -> ~94 us floor.
"""

import numpy as np

import concourse.mybir as mybir
import concourse.tile as tile
from concourse import bacc
from concourse.bass_utils import run_bass_kernel_spmd

# Problem constants (hardcoded per harness contract).
B, C, H, W = 8, 32, 512, 512
K = 8  # block size
P = 128  # SBUF partitions
NBR = H // K  # 64 block-rows per channel
CPP = P // NBR  # channels per partition-tile (2)
N_CORES = 8
N_CHUNKS = C // CPP  # 16 tiles per core

BETA_INT = 0.2 / 8.0
BETA_EDGE = 0.2 / 5.0
GAMMA_INT = 8.0 / 5.0  # (3*Nr-1)/(2*Nr-1) at Nr=3
GAMMA_EDGE = 5.0 / 3.0  # at Nr=2

_EDGE_PARTS = (0, NBR - 1, NBR, P - 1)  # block-row 0/63 of each channel


def _make_weights():
    """Banded stationary matrices (vertical taps at partition +-1),
    block-diagonal per channel, beta folded in per output partition."""
    beta = np.full(P, BETA_INT, np.float32)
    beta[list(_EDGE_PARTS)] = BETA_EDGE
    gamma = np.full(P, GAMMA_INT, np.float32)
    gamma[list(_EDGE_PARTS)] = GAMMA_EDGE
    wv = np.zeros((P, P), np.float32)  # 0.8*I + beta*band2 (on x)
    wh = np.zeros((P, P), np.float32)  # beta*band3 (on hp)
    wcor = np.zeros((P, P), np.float32)  # (gamma-1)*beta*band2 (on x edges)
    for m in range(P):
        for d in (-1, 0, 1):
            k = m + d
            if not (0 <= k < P and k // NBR == m // NBR):
                continue
            wh[k, m] = beta[m]
            if d == 0:
                wv[k, m] = 0.8
            else:
                wv[k, m] = beta[m]
                wcor[k, m] = (gamma[m] - 1.0) * beta[m]
    return {
        "wv": wv.astype(np.float16),
        "wh": wh.astype(np.float16),
        "wcor": wcor.astype(np.float16),
        "gv": gamma.reshape(P, 1),
    }


def _build_program(n_reps=1):
    f32 = mybir.dt.float32
    f16 = mybir.dt.float16

    nc = bacc.Bacc("TRN2", target_bir_lowering=False, debug=False,
                   num_devices=N_CORES)

    x_dram = nc.dram_tensor("x", [C, H, W], f16, kind="ExternalInput")
    y_dram = nc.dram_tensor("y", [C, H, W], f16, kind="ExternalOutput")
    wv_dram = nc.dram_tensor("wv", [P, P], f16, kind="ExternalInput")
    wh_dram = nc.dram_tensor("wh", [P, P], f16, kind="ExternalInput")
    wcor_dram = nc.dram_tensor("wcor", [P, P], f16, kind="ExternalInput")
    gv_dram = nc.dram_tensor("gv", [P, 1], f32, kind="ExternalInput")

    # partition axis = (channel, block-row); free = (u, w)
    x_v = x_dram[:].rearrange("c (bi u) w -> (c bi) u w", u=K)
    y_v = y_dram[:].rearrange("c (bi u) w -> (c bi) u w", u=K)

    HALF = K // 2  # u-slices per PSUM tile (4 banks)

    with tile.TileContext(nc) as tc:
        with (
            tc.tile_pool(name="wpool", bufs=1) as wpool,
            tc.tile_pool(name="sbuf", bufs=5) as sbuf,
            tc.tile_pool(name="psum", bufs=2, space="PSUM") as psum,
        ):
            # weights ride the store ring (idle at startup) so the first
            # x-chunk load starts immediately on the load ring
            wv_t = wpool.tile([P, P], f16, tag="wv")
            nc.scalar.dma_start(wv_t[:], wv_dram[:])
            wh_t = wpool.tile([P, P], f16, tag="wh")
            nc.scalar.dma_start(wh_t[:], wh_dram[:])
            wcor_t = wpool.tile([P, P], f16, tag="wcor")
            nc.scalar.dma_start(wcor_t[:], wcor_dram[:])
            gv_t = wpool.tile([P, 1], f32, tag="gv")
            nc.scalar.dma_start(gv_t[:], gv_dram[:])

            for _rep in range(n_reps):
                for i in range(N_CHUNKS):
                    p0 = i * P
                    xin = sbuf.tile([P, K, W], f16, tag="xin")
                    # loads stay exclusively on the qSyncDynamicHW ring:
                    # mixing dependent stores into the same FIFO ring
                    # head-of-line-blocks later loads (measured +38us).
                    # chunk 0 loads in halves so the PE starts ~2us sooner.
                    if i == 0:
                        nc.sync.dma_start(
                            xin[:, :HALF, :], x_v[p0 : p0 + P, :HALF, :],
                        )
                        nc.sync.dma_start(
                            xin[:, HALF:, :], x_v[p0 : p0 + P, HALF:, :],
                        )
                    else:
                        nc.sync.dma_start(xin[:], x_v[p0 : p0 + P])

                    # hp[w] = x[w-8] + x[w+8] (edge cols: single neighbor,
                    # gamma-prescaled) - all on DVE so hp has exactly one
                    # producer engine; interior add split per half for
                    # finer pipelining
                    hp = sbuf.tile([P, K, W], f16, tag="hp")
                    for h in range(2):
                        u0 = h * HALF
                        nc.vector.tensor_add(
                            hp[:, u0 : u0 + HALF, K : W - K],
                            xin[:, u0 : u0 + HALF, : W - 2 * K],
                            xin[:, u0 : u0 + HALF, 2 * K :],
                        )
                    nc.vector.tensor_scalar_mul(
                        hp[:, :, :K], xin[:, :, K : 2 * K], gv_t[:],
                    )
                    nc.vector.tensor_scalar_mul(
                        hp[:, :, W - K :], xin[:, :, W - 2 * K : W - K],
                        gv_t[:],
                    )

                    out_t = sbuf.tile([P, K, W], f16, tag="out")
                    for h in range(2):
                        u0 = h * HALF
                        u = psum.tile([P, HALF, W], f32, tag="u")
                        # vertical taps + 0.8*center on x (no hp dep, so
                        # the PE can start as soon as the load lands)
                        for uu in range(HALF):
                            nc.tensor.matmul(
                                u[:, uu, :], wv_t[:], xin[:, u0 + uu, :],
                                start=True, stop=False,
                            )
                        # gamma correction for the vertical taps in the
                        # 8-wide w-edge strips, all 4 u-slices per matmul
                        nc.tensor.matmul(
                            u[:, :, :K], wcor_t[:],
                            xin[:, u0 : u0 + HALF, :K],
                            start=False, stop=False,
                        )
                        nc.tensor.matmul(
                            u[:, :, W - K :], wcor_t[:],
                            xin[:, u0 : u0 + HALF, W - K :],
                            start=False, stop=False,
                        )
                        # horizontal(+diagonal) taps via hp
                        for uu in range(HALF):
                            nc.tensor.matmul(
                                u[:, uu, :], wh_t[:], hp[:, u0 + uu, :],
                                start=False, stop=True,
                            )
                        # downcast PSUM f32 -> SBUF fp16, split 3:1
                        # between ScalarE and DVE so neither co-paces
                        # with the PE's ~4.3us/chunk
                        nc.scalar.copy(
                            out_t[:, u0 : u0 + 3, :], u[:, 0:3, :],
                        )
                        nc.vector.tensor_copy(
                            out_t[:, u0 + 3, :], u[:, 3, :],
                        )
                        # last chunk: store each half as soon as it is
                        # downcast to pull in the pipeline tail
                        if i == N_CHUNKS - 1:
                            nc.scalar.dma_start(
                                y_v[p0 : p0 + P, u0 : u0 + HALF, :],
                                out_t[:, u0 : u0 + HALF, :],
                            )
                    # stores exclusively on the second HWDGE ring
                    # (qScalarDynamicHW) so loads and stores stream
                    # concurrently without blocking each other
                    if i != N_CHUNKS - 1:
                        nc.scalar.dma_start(y_v[p0 : p0 + P], out_t[:])
    nc.compile()
    return nc


_CACHE = {}


def _get_program():
    if "nc" not in _CACHE:
        _CACHE["nc"] = _build_program()
        _CACHE["w"] = _make_weights()
    return _CACHE["nc"], _CACHE["w"]


def run(x, trace=False, **spmd_kwargs):
    """x: [B, C, H, W] f32 -> (results object, output [B, C, H, W] f32)."""
    nc, weights = _get_program()
    x16 = np.ascontiguousarray(x).astype(np.float16)
    in_maps = [{"x": x16[i], **weights} for i in range(N_CORES)]
    res = run_bass_kernel_spmd(nc, in_maps, list(range(N_CORES)),
                               trace=trace, **spmd_kwargs)
    out = np.stack([res.results[i]["y"] for i in range(N_CORES)], axis=0)
    return res, out.astype(np.float32)


def kernel(x, kernel_size=8, **_ignored):
    assert int(kernel_size) == K, f"kernel hardcoded for k={K}"
    x = np.asarray(x)
    assert x.shape == (B, C, H, W), x.shape
    _, out = run(x)
    return out


if __name__ == "__main__":
    rng = np.random.default_rng(0)
    x = rng.standard_normal((B, C, H, W), dtype=np.float32)
    out = kernel(x, 8)
    print("out", out.shape, out.dtype, float(np.abs(out).mean()))


# revision 9
# speedup vs baseline: 1.1519x; 1.1519x over previous
"""Trainium2 Bass kernel for block-neighbor "contamination" stencil.

Problem: x [B=8, C=32, H=512, W=512] f32, kernel_size k=8.
The image is a 64x64 grid of 8x8 blocks. For each block, out = 0.8*block +
0.2 * mean(8 neighboring blocks) elementwise over the 8x8 tile, with
zero-padding of the block grid and per-position valid-neighbor counts
(interior 8, edges 5, corners 3).

Equivalent pixel form: a sparse 3x3 stencil with taps at +-8 pixels:
    out[r,w] = 0.8*x[r,w] + beta(r,w) * nsum[r,w]
    nsum[r,w] = sum over (dr,dw) in {-8,0,8}^2, (dr,dw) != (0,0), of
                x[r+dr, w+dw]  (zero pad at image borders)
    beta(r,w) = 0.2 / count(r,w),  count = Nr*Nw - 1,
    Nr/Nw = 2 at the first/last block row/col, else 3.

Strategy (pure data parallel, 1 batch item per NeuronCore, 8 cores):
  * The kernel is HBM-bandwidth bound (~358 GB/s/core), so all HBM I/O is
    fp16: the host casts x to fp16 before upload and widens y back to f32
    after download. That halves DRAM traffic vs f32 (32 MiB/core total)
    and costs ~5e-4 relative error - far inside the 2e-2 gate.
  * Layout: SBUF partition p = (channel-pair, block-row bi); free dim =
    (u = row-within-block 0..7, w 0..511). One partition = one block-row =
    8 consecutive image rows; a 128-partition chunk is one contiguous
    1 MiB DRAM region.
  * Work is split so no compute engine exceeds ~75us (under the ~100us
    DMA floor):
      - DVE precomputes the horizontal pair-sum hp[w] = x[w-8] + x[w+8]
        in fp16 (2x DVE mode), plus 1/4 of the PSUM->fp16 downcast.
      - PE does 2 banded matmuls per u-slice: wv = 0.8*I + beta*band2
        (vertical taps, partition +-1, block-diagonal per channel) on x,
        and wh = beta*band3 on hp. PSUM accumulates the final value.
      - ScalarE writes hp's 8-wide edge columns as gamma(partition)-
        scaled copies and does 3/4 of the downcast.
  * Block-column edges (first/last 8 columns): true coefficient is
    gamma*beta, handled by (a) gamma-prescaling hp's edge columns and
    (b) a tiny wcor = (gamma-1)*beta*band2 matmul on x's edge columns.
    gamma varies per OUTPUT partition but hp prescaling uses the SOURCE
    partition's gamma; they differ only for cross-block-row taps at
    block-rows 0/1/62/63 (1.6 vs 5/3), adding ~1e-4 norm error. Total
    relative error stays ~6e-4, far inside the 2e-2 gate.
  * Input DMAs ride the qSyncDynamicHW ring, output DMAs the
    qScalarDynamicHW ring, so loads and stores stream concurrently.

HBM traffic per core: 16 MiB in + 16 MiB out @ ~358 GB/s -> ~94 us floor.
"""

import numpy as np

import concourse.mybir as mybir
import concourse.tile as tile
from concourse import bacc
from concourse.bass_utils import run_bass_kernel_spmd

# Problem constants (hardcoded per harness contract).
B, C, H, W = 8, 32, 512, 512
K = 8  # block size
P = 128  # SBUF partitions
NBR = H // K  # 64 block-rows per channel
CPP = P // NBR  # channels per partition-tile (2)
N_CORES = 8
N_CHUNKS = C // CPP  # 16 tiles per core

BETA_INT = 0.2 / 8.0
BETA_EDGE = 0.2 / 5.0
GAMMA_INT = 8.0 / 5.0  # (3*Nr-1)/(2*Nr-1) at Nr=3
GAMMA_EDGE = 5.0 / 3.0  # at Nr=2

_EDGE_PARTS = (0, NBR - 1, NBR, P - 1)  # block-row 0/63 of each channel


def _make_weights():
    """Banded stationary matrices (vertical taps at partition +-1),
    block-diagonal per channel, beta folded in per output partition."""
    beta = np.full(P, BETA_INT, np.float32)
    beta[list(_EDGE_PARTS)] = BETA_EDGE
    gamma = np.full(P, GAMMA_INT, np.float32)
    gamma[list(_EDGE_PARTS)] = GAMMA_EDGE
    wv = np.zeros((P, P), np.float32)  # 0.8*I + beta*band2 (on x)
    wh = np.zeros((P, P), np.float32)  # beta*band3 (on hp)
    wcor = np.zeros((P, P), np.float32)  # (gamma-1)*beta*band2 (on x edges)
    for m in range(P):
        for d in (-1, 0, 1):
            k = m + d
            if not (0 <= k < P and k // NBR == m // NBR):
                continue
            wh[k, m] = beta[m]
            if d == 0:
                wv[k, m] = 0.8
            else:
                wv[k, m] = beta[m]
                wcor[k, m] = (gamma[m] - 1.0) * beta[m]
    return {
        "wv": wv.astype(np.float16),
        "wh": wh.astype(np.float16),
        "wcor": wcor.astype(np.float16),
        "gv": gamma.reshape(P, 1),
    }


def _build_program(n_reps=1):
    f32 = mybir.dt.float32
    f16 = mybir.dt.float16

    nc = bacc.Bacc("TRN2", target_bir_lowering=False, debug=False,
                   num_devices=N_CORES)

    x_dram = nc.dram_tensor("x", [C, H, W], f16, kind="ExternalInput")
    y_dram = nc.dram_tensor("y", [C, H, W], f16, kind="ExternalOutput")
    wv_dram = nc.dram_tensor("wv", [P, P], f16, kind="ExternalInput")
    wh_dram = nc.dram_tensor("wh", [P, P], f16, kind="ExternalInput")
    wcor_dram = nc.dram_tensor("wcor", [P, P], f16, kind="ExternalInput")
    gv_dram = nc.dram_tensor("gv", [P, 1], f32, kind="ExternalInput")

    # partition axis = (channel, block-row); free = (u, w)
    x_v = x_dram[:].rearrange("c (bi u) w -> (c bi) u w", u=K)
    y_v = y_dram[:].rearrange("c (bi u) w -> (c bi) u w", u=K)

    HALF = K // 2  # u-slices per PSUM tile (4 banks)

    with tile.TileContext(nc) as tc:
        with (
            tc.tile_pool(name="wpool", bufs=1) as wpool,
            tc.tile_pool(name="sbuf", bufs=5) as sbuf,
            tc.tile_pool(name="psum", bufs=2, space="PSUM") as psum,
        ):
            # weights ride the store ring (idle at startup) so the first
            # x-chunk load starts immediately on the load ring
            wv_t = wpool.tile([P, P], f16, tag="wv")
            nc.scalar.dma_start(wv_t[:], wv_dram[:])
            wh_t = wpool.tile([P, P], f16, tag="wh")
            nc.scalar.dma_start(wh_t[:], wh_dram[:])
            wcor_t = wpool.tile([P, P], f16, tag="wcor")
            nc.scalar.dma_start(wcor_t[:], wcor_dram[:])
            gv_t = wpool.tile([P, 1], f32, tag="gv")
            nc.scalar.dma_start(gv_t[:], gv_dram[:])

            for _rep in range(n_reps):
                for i in range(N_CHUNKS):
                    p0 = i * P
                    xin = sbuf.tile([P, K, W], f16, tag="xin")
                    # loads stay exclusively on the qSyncDynamicHW ring:
                    # mixing dependent stores into the same FIFO ring
                    # head-of-line-blocks later loads (measured +38us).
                    # chunk 0 loads in halves so the PE starts ~2us sooner.
                    if i == 0:
                        nc.sync.dma_start(
                            xin[:, :HALF, :], x_v[p0 : p0 + P, :HALF, :],
                        )
                        nc.sync.dma_start(
                            xin[:, HALF:, :], x_v[p0 : p0 + P, HALF:, :],
                        )
                    else:
                        nc.sync.dma_start(xin[:], x_v[p0 : p0 + P])

                    # hp[w] = x[w-8] + x[w+8] (edge cols: single neighbor,
                    # gamma-prescaled) - all on DVE so hp has exactly one
                    # producer engine; interior add split per half for
                    # finer pipelining
                    hp = sbuf.tile([P, K, W], f16, tag="hp")
                    for h in range(2):
                        u0 = h * HALF
                        nc.vector.tensor_add(
                            hp[:, u0 : u0 + HALF, K : W - K],
                            xin[:, u0 : u0 + HALF, : W - 2 * K],
                            xin[:, u0 : u0 + HALF, 2 * K :],
                        )
                    nc.vector.tensor_scalar_mul(
                        hp[:, :, :K], xin[:, :, K : 2 * K], gv_t[:],
                    )
                    nc.vector.tensor_scalar_mul(
                        hp[:, :, W - K :], xin[:, :, W - 2 * K : W - K],
                        gv_t[:],
                    )

                    out_t = sbuf.tile([P, K, W], f16, tag="out")
                    for h in range(2):
                        u0 = h * HALF
                        u = psum.tile([P, HALF, W], f32, tag="u")
                        # vertical taps + 0.8*center on x (no hp dep, so
                        # the PE can start as soon as the load lands)
                        for uu in range(HALF):
                            nc.tensor.matmul(
                                u[:, uu, :], wv_t[:], xin[:, u0 + uu, :],
                                start=True, stop=False,
                            )
                        # gamma correction for the vertical taps in the
                        # 8-wide w-edge strips, all 4 u-slices per matmul
                        nc.tensor.matmul(
                            u[:, :, :K], wcor_t[:],
                            xin[:, u0 : u0 + HALF, :K],
                            start=False, stop=False,
                        )
                        nc.tensor.matmul(
                            u[:, :, W - K :], wcor_t[:],
                            xin[:, u0 : u0 + HALF, W - K :],
                            start=False, stop=False,
                        )
                        # horizontal(+diagonal) taps via hp
                        for uu in range(HALF):
                            nc.tensor.matmul(
                                u[:, uu, :], wh_t[:], hp[:, u0 + uu, :],
                                start=False, stop=True,
                            )
                        # downcast PSUM f32 -> SBUF fp16 on ScalarE (one
                        # consumer engine frees the PSUM tile)
                        nc.scalar.copy(
                            out_t[:, u0 : u0 + HALF, :], u[:],
                        )
                        # last chunk: store each half as soon as it is
                        # downcast to pull in the pipeline tail
                        if i == N_CHUNKS - 1:
                            nc.scalar.dma_start(
                                y_v[p0 : p0 + P, u0 : u0 + HALF, :],
                                out_t[:, u0 : u0 + HALF, :],
                            )
                    # stores exclusively on the second HWDGE ring
                    # (qScalarDynamicHW) so loads and stores stream
                    # concurrently without blocking each other
                    if i != N_CHUNKS - 1:
                        nc.scalar.dma_start(y_v[p0 : p0 + P], out_t[:])
    nc.compile()
    return nc


_CACHE = {}


def _get_program():
    if "nc" not in _CACHE:
        _CACHE["nc"] = _build_program()
        _CACHE["w"] = _make_weights()
    return _CACHE["nc"], _CACHE["w"]


def run(x, trace=False, **spmd_kwargs):
    """x: [B, C, H, W] f32 -> (results object, output [B, C, H, W] f32)."""
    nc, weights = _get_program()
    x16 = np.ascontiguousarray(x).astype(np.float16)
    in_maps = [{"x": x16[i], **weights} for i in range(N_CORES)]
    res = run_bass_kernel_spmd(nc, in_maps, list(range(N_CORES)),
                               trace=trace, **spmd_kwargs)
    out = np.stack([res.results[i]["y"] for i in range(N_CORES)], axis=0)
    return res, out.astype(np.float32)


def kernel(x, kernel_size=8, **_ignored):
    assert int(kernel_size) == K, f"kernel hardcoded for k={K}"
    x = np.asarray(x)
    assert x.shape == (B, C, H, W), x.shape
    _, out = run(x)
    return out


if __name__ == "__main__":
    rng = np.random.default_rng(0)
    x = rng.standard_normal((B, C, H, W), dtype=np.float32)
    out = kernel(x, 8)
    print("out", out.shape, out.dtype, float(np.abs(out).mean()))
